# revision 16
# baseline (speedup 1.0000x reference)
"""Trainium2 Bass kernel for nn_Correct_PrototypeManager (segment_reduce).

Reference computation:
    pred_lbl = argmax(preds, axis=1)                      # [B, H, W]
    feats_up = bilinear_resize(feats, H, W)               # [B, C, H, W]
    joint[b,k,h,w] = (masks==k) & (pred_lbl==k)
    counts[b,k] = sum_hw joint ; sums[b,k,c] = sum_hw feats_up * joint
    proto = mean_b( sums / (counts + eps) )               # [K, C]

Algebra: bilinear upsample is linear, feats_up = (Uh (x) Uw) @ feats, so
    sums[k,c] = <U^T joint_k, feats_c>   (downsample the one-hot instead)
counts are preserved exactly because rows of U sum to 1.

Device pipeline (per core, one image per core, batch-parallel over 8):
  1. DVE builds the gated one-hot joint[hf, (k, wf)] in bf16 per hf-half:
     tree-max over classes, then joint = (gt*2^20*preds == maxv*2^20)
     where gt = (mask == k).  The 2^20 scale is a power of two, so bf16
     equality is exact: joint matches the f32-argmax one-hot up to
     bf16-rounding ties of preds.
  2. PE stage 1 contracts hf: A[hc, (k, wf)] = Uh^T @ joint  (bf16).
  3. HW DMA-transpose flips A -> AT[wf, (k, wh, hc)] (xbar, off-PE).
  4. PE stage 2 contracts wf: B[wc, (hc, k)] = Uw^T @ AT  (bf16).
  5. Final: per 128-pixel chunk, stationary B-slice [128, 22] x moving
     feats^T chunk [128, 258] accumulate psum[22, 258]; column 256 of
     feats is ones, so counts fall out of the same matmuls.
Host combines the 8 per-image [22, 257] partials (divide + batch mean).
"""

import numpy as np

B = 8
C = 256
K = 21
K2 = 22          # class dim padded: class 21 is an always-zero pad
HC = WC = 64
HF = WF = 256
EPS = 1e-6
N_CORES = 8
PIX = HC * WC    # 4096
KW = K * WF      # 5376
KW2 = K2 * WF    # 5632 = 11 * 512
HK = HC * K2     # 1408
NCHUNK = PIX // 128  # 32
CF = C + 2       # 258: feats cols + ones col + zero pad
SCALE = 1048576.0    # 2^20, exact in bf16

_PROGRAM_CACHE: dict = {}


def _upsample_matrix(n_in: int, n_out: int) -> np.ndarray:
    """U [n_out, n_in] with resize(x, 'bilinear', half-pixel) == U @ x."""
    U = np.zeros((n_out, n_in), dtype=np.float64)
    scale = n_in / n_out
    for i in range(n_out):
        src = (i + 0.5) * scale - 0.5
        f = int(np.floor(src))
        w = src - f
        lo = min(max(f, 0), n_in - 1)
        hi = min(max(f + 1, 0), n_in - 1)
        U[i, lo] += 1.0 - w
        U[i, hi] += w
    return U.astype(np.float32)


def _build_program(stage: int = 99):
    import concourse.bass as bass
    import concourse.bacc as bacc
    import concourse.tile as tile
    from concourse import mybir
    from contextlib import ExitStack

    f32 = mybir.dt.float32
    bf16 = mybir.dt.bfloat16
    AOT = mybir.AluOpType

    nc = bacc.Bacc("TRN2", target_bir_lowering=False, debug=False,
                   num_devices=N_CORES)

    # preds packed f32 (exact argmax), k-major: [half, hf 128, (k, wf)]
    preds_d = nc.dram_tensor("preds", [2, 128, KW], f32,
                             kind="ExternalInput")
    mask_d = nc.dram_tensor("mask", [2, 128, WF], bf16, kind="ExternalInput")
    # feats^T packed per 128-pixel chunk; col 256 = ones (counts), 257 = 0
    ft_d = nc.dram_tensor("ft", [128, NCHUNK * CF], bf16,
                          kind="ExternalInput")
    u_d = nc.dram_tensor("u", [2, 128, HC], bf16, kind="ExternalInput")
    out_d = nc.dram_tensor("out", [K2, C + 1], f32, kind="ExternalOutput")
    dbg_d = None
    if stage < 99:  # extra debug dump, full pipeline still built
        dbg_d = nc.dram_tensor("dbg", [128, 128], f32, kind="ExternalOutput")

    with tile.TileContext(nc) as tc, ExitStack() as ctx:
        const_pool = ctx.enter_context(tc.tile_pool(name="const", bufs=1))
        joint_pool = ctx.enter_context(tc.tile_pool(name="joint", bufs=1))
        ft_pool = ctx.enter_context(tc.tile_pool(name="ft", bufs=1))
        st_pool = ctx.enter_context(tc.tile_pool(name="st", bufs=1))
        res_pool = ctx.enter_context(tc.tile_pool(name="res", bufs=1))
        ps_pool = ctx.enter_context(
            tc.tile_pool(name="ps", bufs=4, space="PSUM"))
        psb_pool = ctx.enter_context(
            tc.tile_pool(name="psb", bufs=1, space="PSUM"))
        psf_pool = ctx.enter_context(
            tc.tile_pool(name="psf", bufs=1, space="PSUM"))

        # --- constants ---
        u_t = []
        for h in range(2):
            t = const_pool.tile([128, HC], bf16, tag=f"u{h}")
            nc.sync.dma_start(t[:], u_d.ap()[h, :, :])
            u_t.append(t)
        # F[p, k, w] = k  (class iota, for the gt one-hot).  gpsimd builds
        # it; a Vector-engine copy re-homes it so the gt TensorTensor's
        # only cross-engine dep is the mask DMA (TT has 1 sync-wait slot).
        f_raw = const_pool.tile([128, KW2], bf16, tag="fio")
        nc.gpsimd.iota(f_raw[:].rearrange("p (k w) -> p k w", k=K2),
                       pattern=[[1, K2], [0, WF]], base=0,
                       channel_multiplier=0,
                       allow_small_or_imprecise_dtypes=True)
        f_t = const_pool.tile([128, KW2], bf16, tag="fio2")
        nc.vector.tensor_copy(f_t[:], f_raw[:])

        # feats^T: one big contiguous DMA, split across 4 queues
        ft_t = ft_pool.tile([128, NCHUNK * CF], bf16, tag="ftbig")
        for q in range(4):
            nc.sync.dma_start(ft_t[32 * q:32 * (q + 1), :],
                              ft_d.ap()[32 * q:32 * (q + 1), :])

        joint_t = []
        oh_pool = ctx.enter_context(tc.tile_pool(name="oh", bufs=1))
        pin_pool = ctx.enter_context(tc.tile_pool(name="pin", bufs=2))
        for h in range(2):
            preds_t = pin_pool.tile([128, KW], f32, tag="preds")
            nc.sync.dma_start(preds_t[:], preds_d.ap()[h, :, :])
            mask_t = pin_pool.tile([128, WF], bf16, tag="mask")
            nc.sync.dma_start(mask_t[:], mask_d.ap()[h, :, :])

            p3 = preds_t[:].rearrange("p (k w) -> p k w", k=K)

            # gt one-hot: gt[p, k, w] = (mask[p, w] == k)  (bf16, 2x mode)
            gt_t = oh_pool.tile([128, KW2], bf16, tag="gt")
            gt3 = gt_t[:].rearrange("p (k w) -> p k w", k=K2)
            nc.vector.tensor_tensor(
                gt3,
                mask_t[:].unsqueeze(1).to_broadcast([128, K2, WF]),
                f_t[:].rearrange("p (k w) -> p k w", k=K2),
                op=AOT.is_equal)

            # tree max over 21 classes (contiguous f32 slices; exact)
            t10 = oh_pool.tile([128, 10 * WF], f32, tag="t10")
            nc.vector.tensor_tensor(
                t10[:], preds_t[:, 0:10 * WF],
                preds_t[:, 10 * WF:20 * WF], op=AOT.max)
            t5 = oh_pool.tile([128, 5 * WF], f32, tag="t5")
            nc.vector.tensor_tensor(
                t5[:], t10[:, 0:5 * WF], t10[:, 5 * WF:10 * WF],
                op=AOT.max)
            t2 = oh_pool.tile([128, 2 * WF], f32, tag="t2")
            nc.vector.tensor_tensor(
                t2[:], t5[:, 0:2 * WF], t5[:, 2 * WF:4 * WF], op=AOT.max)
            t1 = oh_pool.tile([128, WF], f32, tag="t1")
            nc.vector.tensor_tensor(
                t1[:], t2[:, 0:WF], t2[:, WF:2 * WF], op=AOT.max)
            t1b = oh_pool.tile([128, WF], f32, tag="t1b")
            nc.vector.tensor_tensor(
                t1b[:], t1[:], t5[:, 4 * WF:5 * WF], op=AOT.max)
            mx = oh_pool.tile([128, WF], f32, tag="mx")
            nc.vector.tensor_tensor(
                mx[:], t1b[:], preds_t[:, 20 * WF:21 * WF], op=AOT.max)
            # maxv * 2^20 (exact power-of-two scale)
            mx20 = oh_pool.tile([128, WF], f32, tag="mx20")
            nc.vector.tensor_scalar_mul(mx20[:], mx[:], SCALE)

            # mp = (gt * 2^20) * preds ; joint = (mp == maxv*2^20)
            # Exact: scaling by 2^20 is lossless, gate-off gives 0 which
            # never equals maxv*2^20, so joint == f32-argmax one-hot.
            mp_t = oh_pool.tile([128, KW], f32, tag="mp")
            mp3 = mp_t[:].rearrange("p (k w) -> p k w", k=K)
            nc.vector.scalar_tensor_tensor(
                mp3, gt3[:, 0:K, :], SCALE, p3,
                op0=AOT.mult, op1=AOT.mult)

            jt = joint_pool.tile([128, KW2], bf16, tag=f"joint{h}")
            j3 = jt[:].rearrange("p (k w) -> p k w", k=K2)
            nc.vector.memset(j3[:, K, :], 0.0)
            nc.vector.tensor_tensor(
                j3[:, 0:K, :], mp3,
                mx20[:].unsqueeze(1).to_broadcast([128, K, WF]),
                op=AOT.is_equal)
            joint_t.append(jt)

        if stage == 1:  # debug: dump joint slice via gpsimd (casts to f32)
            nc.gpsimd.dma_start(dbg_d.ap()[:, :], joint_t[0][:, 0:128])

        # ----- stage 1: contract hf.  A[hc, (k, wf)] = Uh^T @ joint -----
        a_t = st_pool.tile([64, KW2], bf16, tag="a")
        for ci in range(11):
            fc = 512 * ci
            ps = ps_pool.tile([64, 512], f32, tag="ps")
            nc.tensor.matmul(ps[:], u_t[0][:], joint_t[0][:, fc:fc + 512],
                             start=True, stop=False)
            nc.tensor.matmul(ps[:], u_t[1][:], joint_t[1][:, fc:fc + 512],
                             start=False, stop=True)
            # psum -> sbuf (cast bf16); single engine so the downstream
            # DMA-transpose waits on one semaphore
            nc.scalar.copy(a_t[:, fc:fc + 512], ps[:])

        if stage == 2:
            nc.gpsimd.dma_start(dbg_d.ap()[0:64, :], a_t[:, 0:128])

        # ----- stage 1.5: transpose A via the DMA xbar (off-PE) -----
        # at_big[p, (k, wh, hc)]: element (p, k, wh, hc) = A[hc, k, wh*128+p]
        at_t = st_pool.tile([128, K2 * 2 * HC], bf16, tag="at")
        nc.sync.dma_start_transpose(
            at_t[:].rearrange("p (j h) -> p j h", h=HC), a_t[:])
        at4 = at_t[:].rearrange("p (k w h) -> p k w h", k=K2, w=2)

        if stage == 3:
            nc.gpsimd.dma_start(dbg_d.ap()[:, :], at_t[:, 0:128])

        # ----- stage 2: contract wf.  Bt[wc, (hc, k)] = Uw^T @ AT -----
        # free dim in hc-major (hc, k) order so the final stage can take
        # contiguous [128, 22] stationary slices per pixel chunk.
        b2 = st_pool.tile([128, HK], bf16, tag="b2")
        hc_cuts = [0, 22, 43, 64]
        psb = []
        for j in range(3):
            h0, h1 = hc_cuts[j], hc_cuts[j + 1]
            ps = psb_pool.tile([64, (h1 - h0) * K2], f32, tag=f"psb{j}")
            psb.append(ps)
        for wh in range(2):
            rhs = at4[:, :, wh, :].transpose([0, 2, 1])  # [p, hc, k]
            for j in range(3):
                h0, h1 = hc_cuts[j], hc_cuts[j + 1]
                nc.tensor.matmul(
                    psb[j][:].rearrange("p (h k) -> p h k", k=K2),
                    u_t[wh][:], rhs[:, h0:h1, :],
                    start=(wh == 0), stop=(wh == 1))
        for j in range(3):
            h0, h1 = hc_cuts[j], hc_cuts[j + 1]
            nc.scalar.copy(b2[0:64, h0 * K2:h1 * K2], psb[j][:])
        # partitions 64-127: B shifted by one hc so a 128-pixel chunk
        # (two hc rows) is a single full-partition stationary slice
        nc.sync.dma_start(b2[64:128, 0:HK - K2], b2[0:64, K2:HK])

        if stage == 4:
            nc.gpsimd.dma_start(dbg_d.ap()[:, :], b2[:, 0:128])

        # ----- final: sums[k, c] (+counts in col 256) -----
        ftv = ft_t[:].rearrange("p (n f) -> p n f", n=NCHUNK)
        psf = psf_pool.tile([K2, CF], f32, tag="fin")
        for ch in range(NCHUNK):
            nc.tensor.matmul(psf[:], b2[:, 2 * ch * K2:(2 * ch + 1) * K2],
                             ftv[:, ch, :],
                             start=(ch == 0), stop=(ch == NCHUNK - 1))
        out_sb = res_pool.tile([K2, C + 1], f32, tag="out")
        nc.scalar.copy(out_sb[:], psf[:, 0:C + 1])
        nc.sync.dma_start(out_d.ap()[:, :], out_sb[:])

    nc.compile()
    return nc


def _get_program(stage: int = 99):
    key = f"nc{stage}"
    if key not in _PROGRAM_CACHE:
        _PROGRAM_CACHE[key] = _build_program(stage)
    return _PROGRAM_CACHE[key]


def _host_inputs(feats, preds, masks):
    import ml_dtypes

    bf16 = ml_dtypes.bfloat16
    U = _upsample_matrix(HC, HF)                      # [256, 64], bf16-exact
    u_pack = U.reshape(2, 128, HC).astype(bf16)

    feats = np.asarray(feats, dtype=np.float32)
    preds = np.asarray(preds, dtype=np.float32)
    masks_f = np.asarray(masks).astype(np.float32)

    # preds: [B, 21, 256, 256] -> [B, 2, 128, (k, wf)] k-major, f32
    pr = preds.reshape(B, K, 2, 128, WF).transpose(0, 2, 3, 1, 4)
    pr_pk = np.ascontiguousarray(pr).reshape(B, 2, 128, KW)

    mio_bf = masks_f.reshape(B, 2, 128, WF).astype(bf16)

    # feats^T per 128-pixel chunk: ft[b, p, ch, c]; pix = hc*64 + wc,
    # chunk ch covers hc rows (2ch, 2ch+1); col 256 = 1.0, col 257 = 0.
    ftp = np.zeros((B, 128, NCHUNK, CF), dtype=bf16)
    f4 = feats.reshape(B, C, NCHUNK, 2, WC).transpose(0, 3, 4, 2, 1)
    ftp[..., :C] = f4.reshape(B, 128, NCHUNK, C).astype(bf16)
    ftp[..., C] = 1.0
    ftp = ftp.reshape(B, 128, NCHUNK * CF)

    in_maps = []
    for b in range(B):
        in_maps.append({
            "preds": np.ascontiguousarray(pr_pk[b]),
            "mask": np.ascontiguousarray(mio_bf[b]),
            "ft": np.ascontiguousarray(ftp[b]),
            "u": u_pack,
        })
    return in_maps


def kernel(feats, preds, masks, _results_hook=None):
    from concourse.bass_utils import run_bass_kernel_spmd

    nc = _get_program()
    in_maps = _host_inputs(feats, preds, masks)
    res = run_bass_kernel_spmd(nc, in_maps, list(range(N_CORES)))
    if _results_hook is not None:
        _results_hook(res)

    protos = []
    for b in range(B):
        out = res.results[b]["out"]            # [22, 257] f32
        sums = out[:K, :C]                     # [K, C]
        counts = out[:K, C]                    # [K]
        protos.append(sums / (counts + EPS)[:, None])
    return np.mean(np.stack(protos), axis=0).astype(np.float32)
